# revision 33
# baseline (speedup 1.0000x reference)
"""DenseMoE Trainium2 kernel: nn_DenseMoE_7722351198590.

Reference computation (B=8192, E=16, D=128, U=256, F=256, G=4, K=6, T=0.01):
  eo   = leaky_relu(einsum('bed,edu->bue', x, Wexp) + bias, 0.2)   # [B,U,E]
  g    = relu(einsum('bf,gfe->bge', feat, Wg) + gbias) * gw
  p    = softmax(g/T); top6; w = softmax(p_top6/T)
  out  = einsum('bguk,bgk->bgu', eo_sel, w).reshape(B, G*U)
Returns (out [B, G*U], eo [B, U, E]).

Sharding: data-parallel over batch across 8 NeuronCores (1024 rows each),
weights replicated. Device computes eo in [b, e, u] layout and out in
[b, (g,u)]; host transposes eo to [B, U, E].

Top-k selection notes: selection is done on the unnormalized softmax
numerators eex = exp((g' - max)/T) (same order as p = eex/Z). The DVE
max8 + match_replace pair replicates jax.lax.top_k's lowest-index-first
tie-breaking exactly (match_replace zaps the first unmatched occurrence of
each of the top values, scanning left to right). Final weights use
w_e ∝ exp((eex_e - eex_max) * (100/Z)) over the selected set, matching the
reference's softmax(p_sel/T) to ~1e-5.
"""
import sys

sys.path.insert(0, "/opt/trn_rl_repo")

import numpy as np

import concourse.bass as bass
import concourse.mybir as mybir
import concourse.tile as tile
from concourse.masks import make_identity
from concourse.vector_clock import ScopedClock, VectorClock

F32 = mybir.dt.float32
AF = mybir.ActivationFunctionType
ALU = mybir.AluOpType

B = 8192
E = 16
D = 128
U = 256
F = 256
G = 4
K = 6
N_CORES = 8
B_LOC = B // N_CORES          # 1024
N_CHUNK = B_LOC // 128        # 8
INV_TEMP = 100.0
LEAKY = 0.2

# ---------------------------------------------------------------------------
# Walrus in this container accepts only ONE sem-wait per instruction; Tile
# attaches several. Split them onto single-wait NoOps on the same engine.
# ---------------------------------------------------------------------------
_orig_commit = tile.TileContext._commit_instruction


def _patched_commit(self, inst, lazy_reg_writes: bool = True):
    si = inst.sync_info
    if si is not None and si.on_wait and len(si.on_wait) > 1:
        waits = list(si.on_wait)
        nc = self.nc
        for w in waits[:-1]:
            nop = mybir.InstNoOp(
                name=nc.get_next_instruction_name(), ins=[], outs=[]
            )
            nop.engine = inst.engine
            nop.sync_info = mybir.SyncInfo(on_wait=[w], on_update=[])
            _orig_commit(self, nop, lazy_reg_writes)
        si.on_wait = waits[-1:]
        inst.sync_info = si
    return _orig_commit(self, inst, lazy_reg_writes)


def _patched_drain_and_barrier(self, tick_clock, wait_clock):
    nc = self.nc
    gc = tick_clock.global_clock
    n = len(gc)
    for proc in range(n):
        t = gc[proc]
        if t <= 0:
            continue
        vec = [0] * n
        vec[proc] = t
        nop = nc.sync.nop(hint=f"drain_split_p{proc}", nofuse=True)
        wait_clock.add_sem_waits(nop.ins, ScopedClock({None: VectorClock(vec)}))
        w = nop.ins.sync_info
        if w is not None and w.on_wait and len(w.on_wait) > 1:
            waits = list(w.on_wait)
            w.on_wait = waits[:1]
            nop.ins.sync_info = w
            for extra in waits[1:]:
                nop2 = nc.sync.nop(hint=f"drain_split_p{proc}x", nofuse=True)
                nop2.ins.sync_info = mybir.SyncInfo(on_wait=[extra], on_update=[])
    nc.sync.drain()
    nc.all_engine_barrier()
    assert self.sems is not None
    popped = nc._tile_sem_poison_stack.pop()
    assert popped is self._sem_poison
    nc.clear_and_free_semaphores(list(self.sems.allocated().values()))
    nc.all_engine_barrier()


tile.TileContext._commit_instruction = _patched_commit
tile.TileContext._drain_and_barrier = _patched_drain_and_barrier


# ---------------------------------------------------------------------------
# Kernel builder
# ---------------------------------------------------------------------------

def build_nc(with_expert_bias: bool, with_gating_bias: bool,
             act_leaky: bool = True, fast: bool = True):
    """fast=True: f32r expert matmuls (~1e-4 scale-rel error on eo) and the
    top-k combine as fp16 PE matmuls (eo stationary, per-gate diagonal weight
    matrices as the moving operand, PSUM-accumulated over experts, transposed
    output). fast=False: full-fp32 everywhere with the combine as DVE FMAs."""
    F16 = mybir.dt.float16
    F32R = mybir.dt.float32r
    nc = bass.Bass("TRN2", target_bir_lowering=False, debug=False,
                   num_devices=N_CORES)
    x_d = nc.dram_tensor("x", [B_LOC, E * D], F32, kind="ExternalInput")
    ft_d = nc.dram_tensor("ft", [B_LOC, F], F32, kind="ExternalInput")
    ek_d = nc.dram_tensor("ek", [E, D, U], F32, kind="ExternalInput")
    gk_d = nc.dram_tensor("gk", [G, F, E], F32, kind="ExternalInput")
    gw_d = nc.dram_tensor("gw", [E], F32, kind="ExternalInput")
    eb_d = (nc.dram_tensor("eb", [E, U], F32, kind="ExternalInput")
            if with_expert_bias else None)
    gb_d = (nc.dram_tensor("gb", [G, E], F32, kind="ExternalInput")
            if with_gating_bias else None)
    eo_d = nc.dram_tensor("eo", [B_LOC, E, U], F32, kind="ExternalOutput")
    if fast:
        # transposed combine output: co_t[uh, u, g, b]
        co_d = nc.dram_tensor("co", [2, 128, G, B_LOC], F32,
                              kind="ExternalOutput")
    else:
        co_d = nc.dram_tensor("co", [B_LOC, G * U], F32, kind="ExternalOutput")

    with tile.TileContext(nc) as tc:
        with tc.tile_pool(name="const", bufs=1) as cpool:
            # Expert weights: [d, e, u]
            W_sb = cpool.tile([128, E, U], F32)
            nc.sync.dma_start(out=W_sb[:], in_=ek_d.rearrange("e d u -> d e u"))
            # Gating weights: [f_low, kf, g, e]
            Wg_sb = cpool.tile([128, 2, G, E], F32)
            for a in range(2):
                nc.sync.dma_start(
                    out=Wg_sb[:, a],
                    in_=gk_d.rearrange("g (a p) e -> a p g e", a=2)[a])
            # identity for PE transposes
            I_sb = cpool.tile([128, 128], F32)
            make_identity(nc, I_sb[:])
            W_r = None
            I_h = None
            if fast:
                # f32r-rounded expert weights for the fast matmul path
                W_r = cpool.tile([128, E, U], F32R)
                nc.vector.tensor_copy(
                    W_r.rearrange("p e u -> p (e u)"),
                    W_sb.rearrange("p e u -> p (e u)"))
                # fp16 identity for diagonal-weight construction
                I_h = cpool.tile([128, 128], F16)
                nc.vector.tensor_copy(I_h[:], I_sb[:])
            # broadcast global_weights to all partitions via K=1 matmul
            ones1 = cpool.tile([1, 128], F32)
            nc.vector.memset(ones1[:], 1.0)
            gw1 = cpool.tile([1, E], F32)
            nc.sync.dma_start(out=gw1[:], in_=gw_d.rearrange("(o e) -> o e", o=1))
            gwb = cpool.tile([128, E], F32)
            gbb = None
            ebb = None
            with tc.tile_pool(name="cpsum", bufs=1, space="PSUM") as cpsum:
                gw_ps = cpsum.tile([128, E], F32)
                nc.tensor.matmul(gw_ps[:], ones1[:], gw1[:], start=True,
                                 stop=True)
                nc.vector.tensor_copy(gwb[:], gw_ps[:])
                if with_gating_bias:
                    gb1 = cpool.tile([1, G * E], F32)
                    nc.sync.dma_start(out=gb1[:],
                                      in_=gb_d.rearrange("g e -> (g e)")
                                      .rearrange("(o q) -> o q", o=1))
                    gbb = cpool.tile([128, G * E], F32)
                    gb_ps = cpsum.tile([128, G * E], F32)
                    nc.tensor.matmul(gb_ps[:], ones1[:], gb1[:], start=True,
                                     stop=True)
                    nc.vector.tensor_copy(gbb[:], gb_ps[:])
                if with_expert_bias:
                    eb1 = cpool.tile([1, E * U], F32)
                    nc.sync.dma_start(out=eb1[:],
                                      in_=eb_d.rearrange("e u -> (e u)")
                                      .rearrange("(o q) -> o q", o=1))
                    ebb = cpool.tile([128, E, U], F32)
                    for h in range(E * U // 512):
                        eb_ps = cpsum.tile([128, 512], F32)
                        nc.tensor.matmul(
                            eb_ps[:], ones1[:],
                            eb1[:, h * 512:(h + 1) * 512], start=True,
                            stop=True)
                        nc.vector.tensor_copy(
                            ebb.rearrange("p e u -> p (e u)")
                            [:, h * 512:(h + 1) * 512],
                            eb_ps[:])

            with (
                tc.tile_pool(name="xin", bufs=4) as xpool,
                tc.tile_pool(name="fin", bufs=4) as fpool,
                tc.tile_pool(name="xt", bufs=10) as xtpool,
                tc.tile_pool(name="eo", bufs=3) as eopool,
                tc.tile_pool(name="acc", bufs=3) as accpool,
                tc.tile_pool(name="gat", bufs=3) as gatpool,
                tc.tile_pool(name="ps_t", bufs=1, space="PSUM") as ps_t,
                tc.tile_pool(name="ps_e", bufs=2 if fast else 4,
                             space="PSUM") as ps_e,
                tc.tile_pool(name="ps_g", bufs=1, space="PSUM") as ps_g,
                tc.tile_pool(name="ps_c", bufs=2, space="PSUM") as ps_c,
            ):
                for c in range(N_CHUNK):
                    row = slice(c * 128, (c + 1) * 128)
                    x_sb = xpool.tile([128, E, D], F32)
                    nc.sync.dma_start(
                        out=x_sb[:],
                        in_=x_d[row, :].rearrange("b (e d) -> b e d", e=E))
                    f_sb = fpool.tile([128, F], F32)
                    nc.sync.dma_start(out=f_sb[:], in_=ft_d[row, :])

                    # ---- gating matmul: g[b, (g,e)] ----
                    ft_ps = ps_g.tile([128, F], F32, tag="gps")
                    for k in range(2):
                        nc.tensor.transpose(
                            ft_ps[:, k * 128:(k + 1) * 128],
                            f_sb[:, k * 128:(k + 1) * 128], I_sb[:])
                    ftT = gatpool.tile([128, F], F32, tag="ftT")
                    nc.scalar.activation(ftT[:], ft_ps[:], AF.Copy)
                    g_ps = ps_g.tile([128, G * E], F32, tag="gps")
                    for k in range(2):
                        nc.tensor.matmul(
                            g_ps[:], ftT[:, k * 128:(k + 1) * 128],
                            Wg_sb[:, k, :, :].rearrange("p g e -> p (g e)"),
                            start=(k == 0), stop=(k == 1))

                    # ---- gating vector pipeline ----
                    gat = gatpool.tile([128, G, E], F32, tag="grelu")
                    gwb_b = gwb.rearrange("p (o e) -> p o e", o=1) \
                        .to_broadcast([128, G, E])
                    if with_gating_bias:
                        nc.vector.tensor_tensor(
                            out=gat.rearrange("p g e -> p (g e)"),
                            in0=g_ps[:], in1=gbb[:], op=ALU.add)
                        nc.vector.scalar_tensor_tensor(
                            out=gat[:], in0=gat[:], scalar=0.0, in1=gwb_b,
                            op0=ALU.max, op1=ALU.mult)
                    else:
                        nc.vector.scalar_tensor_tensor(
                            out=gat[:],
                            in0=g_ps.rearrange("p (g e) -> p g e", g=G),
                            scalar=0.0, in1=gwb_b, op0=ALU.max, op1=ALU.mult)
                    m4 = gatpool.tile([128, G], F32, tag="m4")
                    nc.vector.tensor_reduce(
                        out=m4[:], in_=gat[:], axis=mybir.AxisListType.X,
                        op=ALU.max)
                    gs = gatpool.tile([128, G, E], F32, tag="gs")
                    nc.vector.tensor_tensor(
                        out=gs[:], in0=gat[:],
                        in1=m4.rearrange("p (g o) -> p g o", o=1)
                        .to_broadcast([128, G, E]),
                        op=ALU.subtract)
                    eex = gatpool.tile([128, G, E], F32, tag="eex")
                    nc.scalar.activation(eex[:], gs[:], AF.Exp, scale=INV_TEMP)
                    Z4 = gatpool.tile([128, G], F32, tag="Z4")
                    nc.vector.tensor_reduce(
                        out=Z4[:], in_=eex[:], axis=mybir.AxisListType.X,
                        op=ALU.add)
                    rs4 = gatpool.tile([128, G], F32, tag="rs4")
                    nc.vector.reciprocal(rs4[:], Z4[:])
                    nc.vector.tensor_scalar_mul(rs4[:], rs4[:], INV_TEMP)
                    # top-6 selection per gate
                    mx8 = gatpool.tile([128, G, 8], F32, tag="mx8")
                    for g in range(G):
                        nc.vector.max(out=mx8[:, g, :], in_=eex[:, g, :])
                    nc.vector.memset(mx8[:, :, K:8], -1.0)
                    zap = gatpool.tile([128, G, E], F32, tag="zap")
                    for g in range(G):
                        nc.vector.match_replace(
                            out=zap[:, g, :], in_to_replace=mx8[:, g, :],
                            in_values=eex[:, g, :], imm_value=-1.0)
                    mask = gatpool.tile([128, G, E], F32, tag="mask")
                    nc.vector.tensor_scalar(
                        out=mask[:], in0=zap[:], scalar1=0.0, scalar2=None,
                        op0=ALU.is_lt)
                    # w = exp((eex - mx0) * (100/Z)) * mask, then normalize
                    arg = gatpool.tile([128, G, E], F32, tag="arg")
                    nc.vector.tensor_tensor(
                        out=arg[:], in0=eex[:],
                        in1=mx8[:, :, 0:1].to_broadcast([128, G, E]),
                        op=ALU.subtract)
                    nc.vector.tensor_tensor(
                        out=arg[:], in0=arg[:],
                        in1=rs4.rearrange("p (g o) -> p g o", o=1)
                        .to_broadcast([128, G, E]),
                        op=ALU.mult)
                    ew = gatpool.tile([128, G, E], F32, tag="ew")
                    nc.scalar.activation(ew[:], arg[:], AF.Exp)
                    wm = gatpool.tile([128, G, E], F32, tag="wm")
                    nc.vector.tensor_tensor(
                        out=wm[:], in0=ew[:], in1=mask[:], op=ALU.mult)
                    Zw = gatpool.tile([128, G], F32, tag="Zw")
                    nc.vector.tensor_reduce(
                        out=Zw[:], in_=wm[:], axis=mybir.AxisListType.X,
                        op=ALU.add)
                    rw = gatpool.tile([128, G], F32, tag="rw")
                    nc.vector.reciprocal(rw[:], Zw[:])
                    w_sb = gatpool.tile([128, G, E], F32, tag="w", bufs=8)
                    nc.vector.tensor_tensor(
                        out=w_sb[:], in0=wm[:],
                        in1=rw.rearrange("p (g o) -> p g o", o=1)
                        .to_broadcast([128, G, E]),
                        op=ALU.mult)

                    # ---- experts ----
                    eo_sb = eopool.tile([128, E, U], F32)
                    eo_h = (eopool.tile([128, E, U], F16, name="eo_h", tag="eo_h")
                            if fast else None)
                    ct_ps = [ps_c.tile([128, G * 128], F32, name=f"ct{u}",
                                       tag=f"ct{u}")
                             for u in range(2)] if fast else None
                    w_h = None
                    if fast:
                        # fp16 weights for the GPSIMD diagonal builder
                        w_h = gatpool.tile([128, G, E], F16, name="w_h",
                                           tag="w_h", bufs=8)
                        nc.vector.tensor_copy(
                            w_h.rearrange("p g e -> p (g e)"),
                            w_sb.rearrange("p g e -> p (g e)"))
                    xt_dt = F32R if fast else F32
                    W_use = W_r if fast else W_sb
                    xt4_list = []
                    for q in range(E // 4):
                        # 4 transposes into one PSUM bank, one batched copy
                        xt_ps = ps_t.tile([128, 4, 128], F32, name=f"xtp{q}",
                                          tag="xt_ps")
                        for j in range(4):
                            nc.tensor.transpose(
                                xt_ps[:, j, :], x_sb[:, q * 4 + j, :], I_sb[:])
                        xt4 = xtpool.tile([128, 4, 128], xt_dt, name=f"xt4{q}",
                                          tag="xt4")
                        nc.scalar.activation(
                            xt4.rearrange("p j d -> p (j d)"),
                            xt_ps.rearrange("p j d -> p (j d)"), AF.Copy)
                        xt4_list.append(xt4)
                    for q in range(E // 4):
                        for h in range(2):
                            e0 = q * 4 + 2 * h
                            z_ps = ps_e.tile([128, 2, U], F32, name=f"z{e0}",
                                             tag="z_ps")
                            for j in range(2):
                                e = e0 + j
                                nc.tensor.matmul(
                                    z_ps[:, j, :], xt4_list[q][:, 2 * h + j, :],
                                    W_use[:, e, :], start=True, stop=True)
                            if with_expert_bias:
                                nc.vector.tensor_tensor(
                                    out=z_ps.rearrange("p j u -> p (j u)"),
                                    in0=z_ps.rearrange("p j u -> p (j u)"),
                                    in1=ebb[:, e0:e0 + 2, :]
                                    .rearrange("p j u -> p (j u)"),
                                    op=ALU.add)
                            # leaky relu from PSUM, two experts per op
                            if act_leaky:
                                nc.scalar.activation(
                                    eo_sb[:, e0:e0 + 2, :]
                                    .rearrange("p j u -> p (j u)"),
                                    z_ps.rearrange("p j u -> p (j u)"),
                                    AF.Prelu, alpha=LEAKY)
                            else:
                                # sim fallback (CoreSim lacks Prelu); two ops
                                tmp = xtpool.tile([128, 2, U], F32,
                                                  tag="lk_tmp")
                                nc.vector.tensor_scalar_mul(
                                    tmp.rearrange("p j u -> p (j u)"),
                                    z_ps.rearrange("p j u -> p (j u)"), LEAKY)
                                nc.vector.tensor_tensor(
                                    out=eo_sb[:, e0:e0 + 2, :]
                                    .rearrange("p j u -> p (j u)"),
                                    in0=tmp.rearrange("p j u -> p (j u)"),
                                    in1=z_ps.rearrange("p j u -> p (j u)"),
                                    op=ALU.max)
                        if q % 2 == 1:
                            nc.sync.dma_start(
                                out=eo_d[row, (q - 1) * 4:(q + 1) * 4, :],
                                in_=eo_sb[:, (q - 1) * 4:(q + 1) * 4, :])
                        if fast:
                            # one batched fp16 cast for 4 experts
                            nc.vector.tensor_copy(
                                eo_h[:, q * 4:(q + 1) * 4, :]
                                .rearrange("p e u -> p (e u)"),
                                eo_sb[:, q * 4:(q + 1) * 4, :]
                                .rearrange("p e u -> p (e u)"))
                            for j in range(4):
                                e = q * 4 + j
                                dw = xtpool.tile([128, G, 128], F16, tag="dw")
                                for g in range(G):
                                    if g < 1 or (g == 1 and e % 2 == 0):
                                        nc.vector.tensor_scalar(
                                            out=dw[:, g, :], in0=I_h[:],
                                            scalar1=w_sb[:, g, e:e + 1],
                                            scalar2=None, op0=ALU.mult)
                                    else:
                                        nc.gpsimd.affine_select(
                                            out=dw[:, g, :],
                                            in_=w_h[:, g, e:e + 1]
                                            .to_broadcast([128, 128]),
                                            compare_op=ALU.is_equal, fill=0.0,
                                            base=0, pattern=[[-1, 128]],
                                            channel_multiplier=1)
                                for uh in range(2):
                                    nc.tensor.matmul(
                                        ct_ps[uh][:],
                                        eo_h[:, e, uh * 128:(uh + 1) * 128],
                                        dw.rearrange("p g b -> p (g b)"),
                                        start=(e == 0), stop=(e == E - 1))

                    if fast:
                        # ct_ps[uh][u, (g, b)] = sum_e w[b,g,e]*eo[b,e,uh*128+u]
                        for uh in range(2):
                            ct_sb = accpool.tile([128, G * 128], F32)
                            nc.scalar.activation(ct_sb[:], ct_ps[uh][:],
                                                 AF.Copy)
                            nc.sync.dma_start(
                                out=co_d[uh, :, :, c * 128:(c + 1) * 128],
                                in_=ct_sb.rearrange("p (g b) -> p g b", g=G))
                    else:
                        # combine on DVE: out[b,g,u] = sum_e w[b,g,e]*eo[b,e,u]
                        acc = accpool.tile([128, G, U], F32)
                        for g in range(G):
                            for e in range(E):
                                w_col = w_sb[:, g, e:e + 1]
                                if e == 0:
                                    nc.vector.tensor_scalar_mul(
                                        acc[:, g, :], eo_sb[:, 0, :], w_col)
                                else:
                                    nc.vector.scalar_tensor_tensor(
                                        out=acc[:, g, :], in0=eo_sb[:, e, :],
                                        scalar=w_col, in1=acc[:, g, :],
                                        op0=ALU.mult, op1=ALU.add)
                        nc.sync.dma_start(
                            out=co_d[row, :],
                            in_=acc.rearrange("p g u -> p (g u)"))
    return nc


_NC_CACHE: dict = {}
FAST = True


def _get_nc(with_eb: bool, with_gb: bool):
    key = (with_eb, with_gb, FAST)
    if key not in _NC_CACHE:
        _NC_CACHE[key] = build_nc(with_eb, with_gb, fast=FAST)
    return _NC_CACHE[key]


def kernel(inputs, feature_input, expert_kernels, expert_biases,
           gating_kernels, gating_biases, global_weights, _trace=False):
    from concourse.bass_utils import run_bass_kernel_spmd

    inputs = np.ascontiguousarray(np.asarray(inputs, dtype=np.float32))
    feature_input = np.ascontiguousarray(np.asarray(feature_input, np.float32))
    ek = np.ascontiguousarray(np.asarray(expert_kernels, np.float32))
    eb = np.asarray(expert_biases, np.float32)
    gk = np.ascontiguousarray(np.asarray(gating_kernels, np.float32))
    gb = np.asarray(gating_biases, np.float32)
    gw = np.ascontiguousarray(np.asarray(global_weights, np.float32))

    with_eb = bool(np.any(eb))
    with_gb = bool(np.any(gb))
    nc = _get_nc(with_eb, with_gb)

    in_maps = []
    for c in range(N_CORES):
        row = slice(c * B_LOC, (c + 1) * B_LOC)
        m = {
            "x": np.ascontiguousarray(inputs[row]),
            "ft": np.ascontiguousarray(feature_input[row]),
            "ek": ek,
            "gk": gk,
            "gw": gw,
        }
        if with_eb:
            m["eb"] = np.ascontiguousarray(eb)
        if with_gb:
            m["gb"] = np.ascontiguousarray(gb)
        in_maps.append(m)

    res = run_bass_kernel_spmd(nc, in_maps, core_ids=list(range(N_CORES)),
                               trace=_trace)
    if FAST:
        # co_t [2, 128, G, B_LOC] -> co [B_LOC, G*256]
        co = np.concatenate(
            [res.results[c]["co"].transpose(3, 2, 0, 1).reshape(B_LOC, G * U)
             for c in range(N_CORES)], axis=0)
    else:
        co = np.concatenate([res.results[c]["co"] for c in range(N_CORES)],
                            axis=0)
    eo = np.concatenate([res.results[c]["eo"] for c in range(N_CORES)], axis=0)
    eo = np.ascontiguousarray(eo.transpose(0, 2, 1))  # [B, U, E]
    if _trace:
        kernel._last_results = res
    return co, eo


# revision 46
# speedup vs baseline: 1.3108x; 1.3108x over previous
"""DenseMoE Trainium2 kernel: nn_DenseMoE_7722351198590.

Reference computation (B=8192, E=16, D=128, U=256, F=256, G=4, K=6, T=0.01):
  eo   = leaky_relu(einsum('bed,edu->bue', x, Wexp) + bias, 0.2)   # [B,U,E]
  g    = relu(einsum('bf,gfe->bge', feat, Wg) + gbias) * gw
  p    = softmax(g/T); top6; w = softmax(p_top6/T)
  out  = einsum('bguk,bgk->bgu', eo_sel, w).reshape(B, G*U)
Returns (out [B, G*U], eo [B, U, E]).

Sharding: data-parallel over batch across 8 NeuronCores (1024 rows each),
weights replicated. Device computes eo in [b, e, u] layout and out in
[b, (g,u)]; host transposes eo to [B, U, E].

Top-k selection notes: selection is done on the unnormalized softmax
numerators eex = exp((g' - max)/T) (same order as p = eex/Z). The DVE
max8 + match_replace pair replicates jax.lax.top_k's lowest-index-first
tie-breaking exactly (match_replace zaps the first unmatched occurrence of
each of the top values, scanning left to right). Final weights use
w_e ∝ exp((eex_e - eex_max) * (100/Z)) over the selected set, matching the
reference's softmax(p_sel/T) to ~1e-5.
"""
import sys

sys.path.insert(0, "/opt/trn_rl_repo")

import numpy as np

import concourse.bass as bass
import concourse.mybir as mybir
import concourse.tile as tile
from concourse.masks import make_identity
from concourse.vector_clock import ScopedClock, VectorClock

F32 = mybir.dt.float32
AF = mybir.ActivationFunctionType
ALU = mybir.AluOpType

B = 8192
E = 16
D = 128
U = 256
F = 256
G = 4
K = 6
N_CORES = 8
B_LOC = B // N_CORES          # 1024
N_CHUNK = B_LOC // 128        # 8
INV_TEMP = 100.0
LEAKY = 0.2

# ---------------------------------------------------------------------------
# Walrus in this container accepts only ONE sem-wait per instruction; Tile
# attaches several. Split them onto single-wait NoOps on the same engine.
# ---------------------------------------------------------------------------
_orig_commit = tile.TileContext._commit_instruction


def _patched_commit(self, inst, lazy_reg_writes: bool = True):
    si = inst.sync_info
    if si is not None and si.on_wait and len(si.on_wait) > 1:
        waits = list(si.on_wait)
        nc = self.nc
        for w in waits[:-1]:
            nop = mybir.InstNoOp(
                name=nc.get_next_instruction_name(), ins=[], outs=[]
            )
            nop.engine = inst.engine
            nop.sync_info = mybir.SyncInfo(on_wait=[w], on_update=[])
            _orig_commit(self, nop, lazy_reg_writes)
        si.on_wait = waits[-1:]
        inst.sync_info = si
    return _orig_commit(self, inst, lazy_reg_writes)


def _patched_drain_and_barrier(self, tick_clock, wait_clock):
    nc = self.nc
    gc = tick_clock.global_clock
    n = len(gc)
    for proc in range(n):
        t = gc[proc]
        if t <= 0:
            continue
        vec = [0] * n
        vec[proc] = t
        nop = nc.sync.nop(hint=f"drain_split_p{proc}", nofuse=True)
        wait_clock.add_sem_waits(nop.ins, ScopedClock({None: VectorClock(vec)}))
        w = nop.ins.sync_info
        if w is not None and w.on_wait and len(w.on_wait) > 1:
            waits = list(w.on_wait)
            w.on_wait = waits[:1]
            nop.ins.sync_info = w
            for extra in waits[1:]:
                nop2 = nc.sync.nop(hint=f"drain_split_p{proc}x", nofuse=True)
                nop2.ins.sync_info = mybir.SyncInfo(on_wait=[extra], on_update=[])
    nc.sync.drain()
    nc.all_engine_barrier()
    assert self.sems is not None
    popped = nc._tile_sem_poison_stack.pop()
    assert popped is self._sem_poison
    nc.clear_and_free_semaphores(list(self.sems.allocated().values()))
    nc.all_engine_barrier()


tile.TileContext._commit_instruction = _patched_commit
tile.TileContext._drain_and_barrier = _patched_drain_and_barrier


# ---------------------------------------------------------------------------
# Kernel builder
# ---------------------------------------------------------------------------

def build_nc(with_expert_bias: bool, with_gating_bias: bool,
             act_leaky: bool = True, fast: bool = True):
    """fast=True: f32r expert matmuls (~1e-4 scale-rel error on eo) and the
    top-k combine as fp16 PE matmuls (eo stationary, per-gate diagonal weight
    matrices as the moving operand, PSUM-accumulated over experts, transposed
    output). fast=False: full-fp32 everywhere with the combine as DVE FMAs."""
    F16 = mybir.dt.float16
    F32R = mybir.dt.float32r
    nc = bass.Bass("TRN2", target_bir_lowering=False, debug=False,
                   num_devices=N_CORES)
    x_d = nc.dram_tensor("x", [B_LOC, E * D], F32, kind="ExternalInput")
    ft_d = nc.dram_tensor("ft", [B_LOC, F], F32, kind="ExternalInput")
    ek_d = nc.dram_tensor("ek", [E, D, U], F32, kind="ExternalInput")
    gk_d = nc.dram_tensor("gk", [G, F, E], F32, kind="ExternalInput")
    gw_d = nc.dram_tensor("gw", [E], F32, kind="ExternalInput")
    eb_d = (nc.dram_tensor("eb", [E, U], F32, kind="ExternalInput")
            if with_expert_bias else None)
    gb_d = (nc.dram_tensor("gb", [G, E], F32, kind="ExternalInput")
            if with_gating_bias else None)
    eo_d = nc.dram_tensor("eo", [B_LOC, E, U],
                          mybir.dt.float16 if fast else F32,
                          kind="ExternalOutput")
    if fast:
        # transposed combine output: co_t[uh, u, g, b]
        co_d = nc.dram_tensor("co", [2, 128, G, B_LOC], F32,
                              kind="ExternalOutput")
    else:
        co_d = nc.dram_tensor("co", [B_LOC, G * U], F32, kind="ExternalOutput")

    with tile.TileContext(nc) as tc:
        with tc.tile_pool(name="const", bufs=1) as cpool:
            # Small gating constants first on the scalar DMA queue so the
            # gating chain isn't stalled behind the bulk expert weights.
            gw1 = cpool.tile([1, E], F32)
            nc.scalar.dma_start(out=gw1[:],
                                in_=gw_d.rearrange("(o e) -> o e", o=1))
            # Gating weights: [f_low, kf, g, e]
            Wg_sb = cpool.tile([128, 2, G, E], F32)
            for a in range(2):
                nc.scalar.dma_start(
                    out=Wg_sb[:, a],
                    in_=gk_d.rearrange("g (a p) e -> a p g e", a=2)[a])
            # Expert weights: [d, e, u]
            W_sb = cpool.tile([128, E, U], F32)
            for q4 in range(4):
                nc.scalar.dma_start(
                    out=W_sb[:, q4 * 4:(q4 + 1) * 4, :],
                    in_=ek_d.rearrange("e d u -> d e u")[:, q4 * 4:(q4 + 1) * 4, :])
            # identity for PE transposes
            I_sb = cpool.tile([128, 128], F32)
            make_identity(nc, I_sb[:])
            W_r = None
            I_h = None
            if fast:
                # f32r-rounded expert weights for the fast matmul path
                W_r = cpool.tile([128, E, U], F16)
                for q4 in range(4):
                    nc.vector.tensor_copy(
                        W_r[:, q4 * 4:(q4 + 1) * 4, :]
                        .rearrange("p e u -> p (e u)"),
                        W_sb[:, q4 * 4:(q4 + 1) * 4, :]
                        .rearrange("p e u -> p (e u)"))
                # fp16 identity for diagonal-weight construction
                I_h = cpool.tile([128, 128], F16)
                nc.vector.tensor_copy(I_h[:], I_sb[:])
            # broadcast global_weights to all partitions via K=1 matmul
            ones1 = cpool.tile([1, 128], F32)
            nc.vector.memset(ones1[:], 1.0)
            gwb = cpool.tile([128, E], F32)
            gbb = None
            ebb = None
            with tc.tile_pool(name="cpsum", bufs=1, space="PSUM") as cpsum:
                gw_ps = cpsum.tile([128, E], F32)
                nc.tensor.matmul(gw_ps[:], ones1[:], gw1[:], start=True,
                                 stop=True)
                nc.vector.tensor_copy(gwb[:], gw_ps[:])
                if with_gating_bias:
                    gb1 = cpool.tile([1, G * E], F32)
                    nc.sync.dma_start(out=gb1[:],
                                      in_=gb_d.rearrange("g e -> (g e)")
                                      .rearrange("(o q) -> o q", o=1))
                    gbb = cpool.tile([128, G * E], F32)
                    gb_ps = cpsum.tile([128, G * E], F32)
                    nc.tensor.matmul(gb_ps[:], ones1[:], gb1[:], start=True,
                                     stop=True)
                    nc.vector.tensor_copy(gbb[:], gb_ps[:])
                if with_expert_bias:
                    eb1 = cpool.tile([1, E * U], F32)
                    nc.sync.dma_start(out=eb1[:],
                                      in_=eb_d.rearrange("e u -> (e u)")
                                      .rearrange("(o q) -> o q", o=1))
                    ebb = cpool.tile([128, E, U], F32)
                    for h in range(E * U // 512):
                        eb_ps = cpsum.tile([128, 512], F32)
                        nc.tensor.matmul(
                            eb_ps[:], ones1[:],
                            eb1[:, h * 512:(h + 1) * 512], start=True,
                            stop=True)
                        nc.vector.tensor_copy(
                            ebb.rearrange("p e u -> p (e u)")
                            [:, h * 512:(h + 1) * 512],
                            eb_ps[:])

            with (
                tc.tile_pool(name="xin", bufs=4) as xpool,
                tc.tile_pool(name="fin", bufs=4) as fpool,
                tc.tile_pool(name="xt", bufs=10) as xtpool,
                tc.tile_pool(name="eo", bufs=3) as eopool,
                tc.tile_pool(name="acc", bufs=3) as accpool,
                tc.tile_pool(name="gat", bufs=3) as gatpool,
                tc.tile_pool(name="ps_t", bufs=1, space="PSUM") as ps_t,
                tc.tile_pool(name="ps_e", bufs=2 if fast else 4,
                             space="PSUM") as ps_e,
                tc.tile_pool(name="ps_g", bufs=1, space="PSUM") as ps_g,
                tc.tile_pool(name="ps_c", bufs=2, space="PSUM") as ps_c,
            ):
                for c in range(N_CHUNK):
                    row = slice(c * 128, (c + 1) * 128)
                    f_sb = fpool.tile([128, F], F32)
                    nc.sync.dma_start(out=f_sb[:], in_=ft_d[row, :])
                    x_sb = xpool.tile([128, E, D], F32)
                    nc.sync.dma_start(
                        out=x_sb[:],
                        in_=x_d[row, :].rearrange("b (e d) -> b e d", e=E))

                    # ---- gating matmul: g[b, (g,e)] ----
                    ft_ps = ps_g.tile([128, F], F32, tag="gps")
                    for k in range(2):
                        nc.tensor.transpose(
                            ft_ps[:, k * 128:(k + 1) * 128],
                            f_sb[:, k * 128:(k + 1) * 128], I_sb[:])
                    ftT = gatpool.tile([128, F], F32, tag="ftT")
                    nc.scalar.activation(ftT[:], ft_ps[:], AF.Copy)
                    g_ps = ps_g.tile([128, G * E], F32, tag="gps")
                    for k in range(2):
                        nc.tensor.matmul(
                            g_ps[:], ftT[:, k * 128:(k + 1) * 128],
                            Wg_sb[:, k, :, :].rearrange("p g e -> p (g e)"),
                            start=(k == 0), stop=(k == 1))

                    # ---- gating vector pipeline ----
                    gat = gatpool.tile([128, G, E], F32, tag="grelu")
                    gwb_b = gwb.rearrange("p (o e) -> p o e", o=1) \
                        .to_broadcast([128, G, E])
                    if with_gating_bias:
                        nc.vector.tensor_tensor(
                            out=gat.rearrange("p g e -> p (g e)"),
                            in0=g_ps[:], in1=gbb[:], op=ALU.add)
                        nc.vector.scalar_tensor_tensor(
                            out=gat[:], in0=gat[:], scalar=0.0, in1=gwb_b,
                            op0=ALU.max, op1=ALU.mult)
                    else:
                        nc.vector.scalar_tensor_tensor(
                            out=gat[:],
                            in0=g_ps.rearrange("p (g e) -> p g e", g=G),
                            scalar=0.0, in1=gwb_b, op0=ALU.max, op1=ALU.mult)
                    m4 = gatpool.tile([128, G], F32, tag="m4")
                    nc.vector.tensor_reduce(
                        out=m4[:], in_=gat[:], axis=mybir.AxisListType.X,
                        op=ALU.max)
                    gs = gatpool.tile([128, G, E], F32, tag="gs")
                    nc.vector.tensor_tensor(
                        out=gs[:], in0=gat[:],
                        in1=m4.rearrange("p (g o) -> p g o", o=1)
                        .to_broadcast([128, G, E]),
                        op=ALU.subtract)
                    eex = gatpool.tile([128, G, E], F32, tag="eex")
                    nc.scalar.activation(eex[:], gs[:], AF.Exp, scale=INV_TEMP)
                    Z4 = gatpool.tile([128, G], F32, tag="Z4")
                    nc.vector.tensor_reduce(
                        out=Z4[:], in_=eex[:], axis=mybir.AxisListType.X,
                        op=ALU.add)
                    rs4 = gatpool.tile([128, G], F32, tag="rs4")
                    nc.vector.reciprocal(rs4[:], Z4[:])
                    nc.vector.tensor_scalar_mul(rs4[:], rs4[:], INV_TEMP)
                    # top-6 selection per gate
                    mx8 = gatpool.tile([128, G, 8], F32, tag="mx8")
                    for g in range(G):
                        nc.vector.max(out=mx8[:, g, :], in_=eex[:, g, :])
                    nc.vector.memset(mx8[:, :, K:8], -1.0)
                    zap = gatpool.tile([128, G, E], F32, tag="zap")
                    for g in range(G):
                        nc.vector.match_replace(
                            out=zap[:, g, :], in_to_replace=mx8[:, g, :],
                            in_values=eex[:, g, :], imm_value=-1.0)
                    mask = gatpool.tile([128, G, E], F32, tag="mask")
                    nc.vector.tensor_scalar(
                        out=mask[:], in0=zap[:], scalar1=0.0, scalar2=None,
                        op0=ALU.is_lt)
                    # w = exp((eex - mx0) * (100/Z)) * mask, then normalize
                    arg = gatpool.tile([128, G, E], F32, tag="arg")
                    nc.vector.tensor_tensor(
                        out=arg[:], in0=eex[:],
                        in1=mx8[:, :, 0:1].to_broadcast([128, G, E]),
                        op=ALU.subtract)
                    nc.vector.tensor_tensor(
                        out=arg[:], in0=arg[:],
                        in1=rs4.rearrange("p (g o) -> p g o", o=1)
                        .to_broadcast([128, G, E]),
                        op=ALU.mult)
                    ew = gatpool.tile([128, G, E], F32, tag="ew")
                    nc.scalar.activation(ew[:], arg[:], AF.Exp)
                    wm = gatpool.tile([128, G, E], F32, tag="wm")
                    nc.vector.tensor_tensor(
                        out=wm[:], in0=ew[:], in1=mask[:], op=ALU.mult)
                    Zw = gatpool.tile([128, G], F32, tag="Zw")
                    nc.vector.tensor_reduce(
                        out=Zw[:], in_=wm[:], axis=mybir.AxisListType.X,
                        op=ALU.add)
                    rw = gatpool.tile([128, G], F32, tag="rw")
                    nc.vector.reciprocal(rw[:], Zw[:])
                    w_sb = gatpool.tile([128, G, E], F32, tag="w", bufs=8)
                    nc.vector.tensor_tensor(
                        out=w_sb[:], in0=wm[:],
                        in1=rw.rearrange("p (g o) -> p g o", o=1)
                        .to_broadcast([128, G, E]),
                        op=ALU.mult)

                    # ---- experts ----
                    eo_sb = (None if fast
                             else eopool.tile([128, E, U], F32, name="eo_sb",
                                              tag="eo_sb"))
                    eo_h = (eopool.tile([128, E, U], F16, name="eo_h", tag="eo_h")
                            if fast else None)
                    ct_ps = [ps_c.tile([128, G * 128], F32, name=f"ct{u}",
                                       tag=f"ct{u}")
                             for u in range(2)] if fast else None
                    w_h = None
                    if fast:
                        # fp16 weights for the GPSIMD diagonal builder
                        w_h = gatpool.tile([128, G, E], F16, name="w_h",
                                           tag="w_h", bufs=8)
                        nc.vector.tensor_copy(
                            w_h.rearrange("p g e -> p (g e)"),
                            w_sb.rearrange("p g e -> p (g e)"))
                    xt_dt = F16 if fast else F32
                    W_use = W_r if fast else W_sb
                    xt4_list = []
                    for q in range(E // 4):
                        # 4 transposes into one PSUM bank, one batched copy
                        xt_ps = ps_t.tile([128, 4, 128], F32, name=f"xtp{q}",
                                          tag="xt_ps")
                        for j in range(4):
                            nc.tensor.transpose(
                                xt_ps[:, j, :], x_sb[:, q * 4 + j, :], I_sb[:])
                        xt4 = xtpool.tile([128, 4, 128], xt_dt, name=f"xt4{q}",
                                          tag="xt4")
                        nc.scalar.activation(
                            xt4.rearrange("p j d -> p (j d)"),
                            xt_ps.rearrange("p j d -> p (j d)"), AF.Copy)
                        xt4_list.append(xt4)
                    for q in range(E // 4):
                        for h in range(2):
                            e0 = q * 4 + 2 * h
                            z_ps = ps_e.tile([128, 2, U], F32, name=f"z{e0}",
                                             tag="z_ps")
                            for j in range(2):
                                e = e0 + j
                                nc.tensor.matmul(
                                    z_ps[:, j, :], xt4_list[q][:, 2 * h + j, :],
                                    W_use[:, e, :], start=True, stop=True)
                            if with_expert_bias:
                                nc.vector.tensor_tensor(
                                    out=z_ps.rearrange("p j u -> p (j u)"),
                                    in0=z_ps.rearrange("p j u -> p (j u)"),
                                    in1=ebb[:, e0:e0 + 2, :]
                                    .rearrange("p j u -> p (j u)"),
                                    op=ALU.add)
                            # leaky relu from PSUM, two experts per op;
                            # fast mode writes the fp16 tile directly
                            lk_out = (eo_h if fast else eo_sb)[:, e0:e0 + 2, :]
                            if act_leaky:
                                nc.scalar.activation(
                                    lk_out.rearrange("p j u -> p (j u)"),
                                    z_ps.rearrange("p j u -> p (j u)"),
                                    AF.Prelu, alpha=LEAKY)
                            else:
                                # sim fallback (CoreSim lacks Prelu); two ops
                                tmp = xtpool.tile([128, 2, U], F32,
                                                  tag="lk_tmp")
                                nc.vector.tensor_scalar_mul(
                                    tmp.rearrange("p j u -> p (j u)"),
                                    z_ps.rearrange("p j u -> p (j u)"), LEAKY)
                                nc.vector.tensor_tensor(
                                    out=lk_out.rearrange("p j u -> p (j u)"),
                                    in0=tmp.rearrange("p j u -> p (j u)"),
                                    in1=z_ps.rearrange("p j u -> p (j u)"),
                                    op=ALU.max)
                        nc.sync.dma_start(
                            out=eo_d[row, q * 4:(q + 1) * 4, :],
                            in_=(eo_h if fast else eo_sb)
                            [:, q * 4:(q + 1) * 4, :])
                        if fast:
                            for j in range(4):
                                e = q * 4 + j
                                dw = xtpool.tile([128, G, 128], F16, tag="dw")
                                for g in range(G):
                                    if g < 1 or (g == 1 and e % 2 == 0):
                                        nc.vector.tensor_scalar(
                                            out=dw[:, g, :], in0=I_h[:],
                                            scalar1=w_sb[:, g, e:e + 1],
                                            scalar2=None, op0=ALU.mult)
                                    else:
                                        nc.gpsimd.affine_select(
                                            out=dw[:, g, :],
                                            in_=w_h[:, g, e:e + 1]
                                            .to_broadcast([128, 128]),
                                            compare_op=ALU.is_equal, fill=0.0,
                                            base=0, pattern=[[-1, 128]],
                                            channel_multiplier=1)
                                for uh in range(2):
                                    nc.tensor.matmul(
                                        ct_ps[uh][:],
                                        eo_h[:, e, uh * 128:(uh + 1) * 128],
                                        dw.rearrange("p g b -> p (g b)"),
                                        start=(e == 0), stop=(e == E - 1))

                    if fast:
                        # ct_ps[uh][u, (g, b)] = sum_e w[b,g,e]*eo[b,e,uh*128+u]
                        for uh in range(2):
                            ct_sb = accpool.tile([128, G * 128], F32)
                            nc.scalar.activation(ct_sb[:], ct_ps[uh][:],
                                                 AF.Copy)
                            nc.sync.dma_start(
                                out=co_d[uh, :, :, c * 128:(c + 1) * 128],
                                in_=ct_sb.rearrange("p (g b) -> p g b", g=G))
                    else:
                        # combine on DVE: out[b,g,u] = sum_e w[b,g,e]*eo[b,e,u]
                        acc = accpool.tile([128, G, U], F32)
                        for g in range(G):
                            for e in range(E):
                                w_col = w_sb[:, g, e:e + 1]
                                if e == 0:
                                    nc.vector.tensor_scalar_mul(
                                        acc[:, g, :], eo_sb[:, 0, :], w_col)
                                else:
                                    nc.vector.scalar_tensor_tensor(
                                        out=acc[:, g, :], in0=eo_sb[:, e, :],
                                        scalar=w_col, in1=acc[:, g, :],
                                        op0=ALU.mult, op1=ALU.add)
                        nc.sync.dma_start(
                            out=co_d[row, :],
                            in_=acc.rearrange("p g u -> p (g u)"))
    return nc


_NC_CACHE: dict = {}
FAST = True


def _get_nc(with_eb: bool, with_gb: bool):
    key = (with_eb, with_gb, FAST)
    if key not in _NC_CACHE:
        _NC_CACHE[key] = build_nc(with_eb, with_gb, fast=FAST)
    return _NC_CACHE[key]


def kernel(inputs, feature_input, expert_kernels, expert_biases,
           gating_kernels, gating_biases, global_weights, _trace=False):
    from concourse.bass_utils import run_bass_kernel_spmd

    inputs = np.ascontiguousarray(np.asarray(inputs, dtype=np.float32))
    feature_input = np.ascontiguousarray(np.asarray(feature_input, np.float32))
    ek = np.ascontiguousarray(np.asarray(expert_kernels, np.float32))
    eb = np.asarray(expert_biases, np.float32)
    gk = np.ascontiguousarray(np.asarray(gating_kernels, np.float32))
    gb = np.asarray(gating_biases, np.float32)
    gw = np.ascontiguousarray(np.asarray(global_weights, np.float32))

    with_eb = bool(np.any(eb))
    with_gb = bool(np.any(gb))
    nc = _get_nc(with_eb, with_gb)

    in_maps = []
    for c in range(N_CORES):
        row = slice(c * B_LOC, (c + 1) * B_LOC)
        m = {
            "x": np.ascontiguousarray(inputs[row]),
            "ft": np.ascontiguousarray(feature_input[row]),
            "ek": ek,
            "gk": gk,
            "gw": gw,
        }
        if with_eb:
            m["eb"] = np.ascontiguousarray(eb)
        if with_gb:
            m["gb"] = np.ascontiguousarray(gb)
        in_maps.append(m)

    res = run_bass_kernel_spmd(nc, in_maps, core_ids=list(range(N_CORES)),
                               trace=_trace)
    if FAST:
        # co_t [2, 128, G, B_LOC] -> co [B_LOC, G*256]
        co = np.concatenate(
            [res.results[c]["co"].transpose(3, 2, 0, 1).reshape(B_LOC, G * U)
             for c in range(N_CORES)], axis=0)
    else:
        co = np.concatenate([res.results[c]["co"] for c in range(N_CORES)],
                            axis=0)
    eo = np.concatenate([res.results[c]["eo"] for c in range(N_CORES)], axis=0)
    eo = np.ascontiguousarray(eo.transpose(0, 2, 1).astype(np.float32))
    if _trace:
        kernel._last_results = res
    return co, eo


# revision 48
# speedup vs baseline: 1.4084x; 1.0745x over previous
"""DenseMoE Trainium2 kernel: nn_DenseMoE_7722351198590.

Reference computation (B=8192, E=16, D=128, U=256, F=256, G=4, K=6, T=0.01):
  eo   = leaky_relu(einsum('bed,edu->bue', x, Wexp) + bias, 0.2)   # [B,U,E]
  g    = relu(einsum('bf,gfe->bge', feat, Wg) + gbias) * gw
  p    = softmax(g/T); top6; w = softmax(p_top6/T)
  out  = einsum('bguk,bgk->bgu', eo_sel, w).reshape(B, G*U)
Returns (out [B, G*U], eo [B, U, E]).

Sharding: data-parallel over batch across 8 NeuronCores (1024 rows each),
weights replicated. Device computes eo in [b, e, u] layout and out in
[b, (g,u)]; host transposes eo to [B, U, E].

Top-k selection notes: selection is done on the unnormalized softmax
numerators eex = exp((g' - max)/T) (same order as p = eex/Z). The DVE
max8 + match_replace pair replicates jax.lax.top_k's lowest-index-first
tie-breaking exactly (match_replace zaps the first unmatched occurrence of
each of the top values, scanning left to right). Final weights use
w_e ∝ exp((eex_e - eex_max) * (100/Z)) over the selected set, matching the
reference's softmax(p_sel/T) to ~1e-5.
"""
import sys

sys.path.insert(0, "/opt/trn_rl_repo")

import numpy as np

import concourse.bass as bass
import concourse.mybir as mybir
import concourse.tile as tile
from concourse.masks import make_identity
from concourse.vector_clock import ScopedClock, VectorClock

F32 = mybir.dt.float32
AF = mybir.ActivationFunctionType
ALU = mybir.AluOpType

B = 8192
E = 16
D = 128
U = 256
F = 256
G = 4
K = 6
N_CORES = 8
B_LOC = B // N_CORES          # 1024
N_CHUNK = B_LOC // 128        # 8
INV_TEMP = 100.0
LEAKY = 0.2

# ---------------------------------------------------------------------------
# Walrus in this container accepts only ONE sem-wait per instruction; Tile
# attaches several. Split them onto single-wait NoOps on the same engine.
# ---------------------------------------------------------------------------
_orig_commit = tile.TileContext._commit_instruction


def _patched_commit(self, inst, lazy_reg_writes: bool = True):
    si = inst.sync_info
    if si is not None and si.on_wait and len(si.on_wait) > 1:
        waits = list(si.on_wait)
        nc = self.nc
        for w in waits[:-1]:
            nop = mybir.InstNoOp(
                name=nc.get_next_instruction_name(), ins=[], outs=[]
            )
            nop.engine = inst.engine
            nop.sync_info = mybir.SyncInfo(on_wait=[w], on_update=[])
            _orig_commit(self, nop, lazy_reg_writes)
        si.on_wait = waits[-1:]
        inst.sync_info = si
    return _orig_commit(self, inst, lazy_reg_writes)


def _patched_drain_and_barrier(self, tick_clock, wait_clock):
    nc = self.nc
    gc = tick_clock.global_clock
    n = len(gc)
    for proc in range(n):
        t = gc[proc]
        if t <= 0:
            continue
        vec = [0] * n
        vec[proc] = t
        nop = nc.sync.nop(hint=f"drain_split_p{proc}", nofuse=True)
        wait_clock.add_sem_waits(nop.ins, ScopedClock({None: VectorClock(vec)}))
        w = nop.ins.sync_info
        if w is not None and w.on_wait and len(w.on_wait) > 1:
            waits = list(w.on_wait)
            w.on_wait = waits[:1]
            nop.ins.sync_info = w
            for extra in waits[1:]:
                nop2 = nc.sync.nop(hint=f"drain_split_p{proc}x", nofuse=True)
                nop2.ins.sync_info = mybir.SyncInfo(on_wait=[extra], on_update=[])
    nc.sync.drain()
    nc.all_engine_barrier()
    assert self.sems is not None
    popped = nc._tile_sem_poison_stack.pop()
    assert popped is self._sem_poison
    nc.clear_and_free_semaphores(list(self.sems.allocated().values()))
    nc.all_engine_barrier()


tile.TileContext._commit_instruction = _patched_commit
tile.TileContext._drain_and_barrier = _patched_drain_and_barrier


# ---------------------------------------------------------------------------
# Kernel builder
# ---------------------------------------------------------------------------

def build_nc(with_expert_bias: bool, with_gating_bias: bool,
             act_leaky: bool = True, fast: bool = True):
    """fast=True: f32r expert matmuls (~1e-4 scale-rel error on eo) and the
    top-k combine as fp16 PE matmuls (eo stationary, per-gate diagonal weight
    matrices as the moving operand, PSUM-accumulated over experts, transposed
    output). fast=False: full-fp32 everywhere with the combine as DVE FMAs."""
    F16 = mybir.dt.float16
    F32R = mybir.dt.float32r
    nc = bass.Bass("TRN2", target_bir_lowering=False, debug=False,
                   num_devices=N_CORES)
    x_d = nc.dram_tensor("x", [B_LOC, E * D], F32, kind="ExternalInput")
    ft_d = nc.dram_tensor("ft", [B_LOC, F], F32, kind="ExternalInput")
    ek_d = nc.dram_tensor("ek", [E, D, U], F32, kind="ExternalInput")
    gk_d = nc.dram_tensor("gk", [G, F, E], F32, kind="ExternalInput")
    gw_d = nc.dram_tensor("gw", [E], F32, kind="ExternalInput")
    eb_d = (nc.dram_tensor("eb", [E, U], F32, kind="ExternalInput")
            if with_expert_bias else None)
    gb_d = (nc.dram_tensor("gb", [G, E], F32, kind="ExternalInput")
            if with_gating_bias else None)
    eo_d = nc.dram_tensor("eo", [B_LOC, E, U],
                          mybir.dt.float16 if fast else F32,
                          kind="ExternalOutput")
    if fast:
        # transposed combine output: co_t[uh, u, g, b], fp16 (host upconverts)
        co_d = nc.dram_tensor("co", [2, 128, G, B_LOC], F16,
                              kind="ExternalOutput")
    else:
        co_d = nc.dram_tensor("co", [B_LOC, G * U], F32, kind="ExternalOutput")

    with tile.TileContext(nc) as tc:
        with tc.tile_pool(name="const", bufs=1) as cpool:
            # Expert weights: [d, e, u]
            W_sb = cpool.tile([128, E, U], F32)
            for q4 in range(4):
                nc.scalar.dma_start(
                    out=W_sb[:, q4 * 4:(q4 + 1) * 4, :],
                    in_=ek_d.rearrange("e d u -> d e u")[:, q4 * 4:(q4 + 1) * 4, :])
            # Gating weights: [f_low, kf, g, e]
            Wg_sb = cpool.tile([128, 2, G, E], F32)
            for a in range(2):
                nc.scalar.dma_start(
                    out=Wg_sb[:, a],
                    in_=gk_d.rearrange("g (a p) e -> a p g e", a=2)[a])
            # identity for PE transposes
            I_sb = cpool.tile([128, 128], F32)
            make_identity(nc, I_sb[:])
            W_r = None
            I_h = None
            if fast:
                # f32r-rounded expert weights for the fast matmul path
                W_r = cpool.tile([128, E, U], F16)
                for q4 in range(4):
                    nc.vector.tensor_copy(
                        W_r[:, q4 * 4:(q4 + 1) * 4, :]
                        .rearrange("p e u -> p (e u)"),
                        W_sb[:, q4 * 4:(q4 + 1) * 4, :]
                        .rearrange("p e u -> p (e u)"))
                # fp16 identity for diagonal-weight construction
                I_h = cpool.tile([128, 128], F16)
                nc.vector.tensor_copy(I_h[:], I_sb[:])
            # broadcast global_weights to all partitions via K=1 matmul
            ones1 = cpool.tile([1, 128], F32)
            nc.vector.memset(ones1[:], 1.0)
            gw1 = cpool.tile([1, E], F32)
            nc.scalar.dma_start(out=gw1[:], in_=gw_d.rearrange("(o e) -> o e", o=1))
            gwb = cpool.tile([128, E], F32)
            gbb = None
            ebb = None
            with tc.tile_pool(name="cpsum", bufs=1, space="PSUM") as cpsum:
                gw_ps = cpsum.tile([128, E], F32)
                nc.tensor.matmul(gw_ps[:], ones1[:], gw1[:], start=True,
                                 stop=True)
                nc.vector.tensor_copy(gwb[:], gw_ps[:])
                if with_gating_bias:
                    gb1 = cpool.tile([1, G * E], F32)
                    nc.sync.dma_start(out=gb1[:],
                                      in_=gb_d.rearrange("g e -> (g e)")
                                      .rearrange("(o q) -> o q", o=1))
                    gbb = cpool.tile([128, G * E], F32)
                    gb_ps = cpsum.tile([128, G * E], F32)
                    nc.tensor.matmul(gb_ps[:], ones1[:], gb1[:], start=True,
                                     stop=True)
                    nc.vector.tensor_copy(gbb[:], gb_ps[:])
                if with_expert_bias:
                    eb1 = cpool.tile([1, E * U], F32)
                    nc.sync.dma_start(out=eb1[:],
                                      in_=eb_d.rearrange("e u -> (e u)")
                                      .rearrange("(o q) -> o q", o=1))
                    ebb = cpool.tile([128, E, U], F32)
                    for h in range(E * U // 512):
                        eb_ps = cpsum.tile([128, 512], F32)
                        nc.tensor.matmul(
                            eb_ps[:], ones1[:],
                            eb1[:, h * 512:(h + 1) * 512], start=True,
                            stop=True)
                        nc.vector.tensor_copy(
                            ebb.rearrange("p e u -> p (e u)")
                            [:, h * 512:(h + 1) * 512],
                            eb_ps[:])

            with (
                tc.tile_pool(name="xin", bufs=4) as xpool,
                tc.tile_pool(name="fin", bufs=4) as fpool,
                tc.tile_pool(name="xt", bufs=10) as xtpool,
                tc.tile_pool(name="eo", bufs=3) as eopool,
                tc.tile_pool(name="acc", bufs=3) as accpool,
                tc.tile_pool(name="gat", bufs=3) as gatpool,
                tc.tile_pool(name="ps_t", bufs=1, space="PSUM") as ps_t,
                tc.tile_pool(name="ps_e", bufs=2 if fast else 4,
                             space="PSUM") as ps_e,
                tc.tile_pool(name="ps_g", bufs=1, space="PSUM") as ps_g,
                tc.tile_pool(name="ps_c", bufs=2, space="PSUM") as ps_c,
            ):
                for c in range(N_CHUNK):
                    row = slice(c * 128, (c + 1) * 128)
                    f_sb = fpool.tile([128, F], F32)
                    nc.sync.dma_start(out=f_sb[:], in_=ft_d[row, :])
                    x_sb = xpool.tile([128, E, D], F32)
                    nc.sync.dma_start(
                        out=x_sb[:],
                        in_=x_d[row, :].rearrange("b (e d) -> b e d", e=E))

                    # ---- gating matmul: g[b, (g,e)] ----
                    ft_ps = ps_g.tile([128, F], F32, tag="gps")
                    for k in range(2):
                        nc.tensor.transpose(
                            ft_ps[:, k * 128:(k + 1) * 128],
                            f_sb[:, k * 128:(k + 1) * 128], I_sb[:])
                    ftT = gatpool.tile([128, F], F32, tag="ftT")
                    nc.scalar.activation(ftT[:], ft_ps[:], AF.Copy)
                    g_ps = ps_g.tile([128, G * E], F32, tag="gps")
                    for k in range(2):
                        nc.tensor.matmul(
                            g_ps[:], ftT[:, k * 128:(k + 1) * 128],
                            Wg_sb[:, k, :, :].rearrange("p g e -> p (g e)"),
                            start=(k == 0), stop=(k == 1))

                    # ---- gating vector pipeline ----
                    gat = gatpool.tile([128, G, E], F32, tag="grelu")
                    gwb_b = gwb.rearrange("p (o e) -> p o e", o=1) \
                        .to_broadcast([128, G, E])
                    if with_gating_bias:
                        nc.vector.tensor_tensor(
                            out=gat.rearrange("p g e -> p (g e)"),
                            in0=g_ps[:], in1=gbb[:], op=ALU.add)
                        nc.vector.scalar_tensor_tensor(
                            out=gat[:], in0=gat[:], scalar=0.0, in1=gwb_b,
                            op0=ALU.max, op1=ALU.mult)
                    else:
                        nc.vector.scalar_tensor_tensor(
                            out=gat[:],
                            in0=g_ps.rearrange("p (g e) -> p g e", g=G),
                            scalar=0.0, in1=gwb_b, op0=ALU.max, op1=ALU.mult)
                    m4 = gatpool.tile([128, G], F32, tag="m4")
                    nc.vector.tensor_reduce(
                        out=m4[:], in_=gat[:], axis=mybir.AxisListType.X,
                        op=ALU.max)
                    gs = gatpool.tile([128, G, E], F32, tag="gs")
                    nc.vector.tensor_tensor(
                        out=gs[:], in0=gat[:],
                        in1=m4.rearrange("p (g o) -> p g o", o=1)
                        .to_broadcast([128, G, E]),
                        op=ALU.subtract)
                    eex = gatpool.tile([128, G, E], F32, tag="eex")
                    nc.scalar.activation(eex[:], gs[:], AF.Exp, scale=INV_TEMP)
                    Z4 = gatpool.tile([128, G], F32, tag="Z4")
                    nc.vector.tensor_reduce(
                        out=Z4[:], in_=eex[:], axis=mybir.AxisListType.X,
                        op=ALU.add)
                    rs4 = gatpool.tile([128, G], F32, tag="rs4")
                    nc.vector.reciprocal(rs4[:], Z4[:])
                    nc.vector.tensor_scalar_mul(rs4[:], rs4[:], INV_TEMP)
                    # top-6 selection per gate
                    mx8 = gatpool.tile([128, G, 8], F32, tag="mx8")
                    for g in range(G):
                        nc.vector.max(out=mx8[:, g, :], in_=eex[:, g, :])
                    nc.vector.memset(mx8[:, :, K:8], -1.0)
                    zap = gatpool.tile([128, G, E], F32, tag="zap")
                    for g in range(G):
                        nc.vector.match_replace(
                            out=zap[:, g, :], in_to_replace=mx8[:, g, :],
                            in_values=eex[:, g, :], imm_value=-1.0)
                    mask = gatpool.tile([128, G, E], F32, tag="mask")
                    nc.vector.tensor_scalar(
                        out=mask[:], in0=zap[:], scalar1=0.0, scalar2=None,
                        op0=ALU.is_lt)
                    # w = exp((eex - mx0) * (100/Z)) * mask, then normalize
                    arg = gatpool.tile([128, G, E], F32, tag="arg")
                    nc.vector.tensor_tensor(
                        out=arg[:], in0=eex[:],
                        in1=mx8[:, :, 0:1].to_broadcast([128, G, E]),
                        op=ALU.subtract)
                    nc.vector.tensor_tensor(
                        out=arg[:], in0=arg[:],
                        in1=rs4.rearrange("p (g o) -> p g o", o=1)
                        .to_broadcast([128, G, E]),
                        op=ALU.mult)
                    ew = gatpool.tile([128, G, E], F32, tag="ew")
                    nc.scalar.activation(ew[:], arg[:], AF.Exp)
                    wm = gatpool.tile([128, G, E], F32, tag="wm")
                    nc.vector.tensor_tensor(
                        out=wm[:], in0=ew[:], in1=mask[:], op=ALU.mult)
                    Zw = gatpool.tile([128, G], F32, tag="Zw")
                    nc.vector.tensor_reduce(
                        out=Zw[:], in_=wm[:], axis=mybir.AxisListType.X,
                        op=ALU.add)
                    rw = gatpool.tile([128, G], F32, tag="rw")
                    nc.vector.reciprocal(rw[:], Zw[:])
                    w_sb = gatpool.tile([128, G, E], F32, tag="w", bufs=8)
                    nc.vector.tensor_tensor(
                        out=w_sb[:], in0=wm[:],
                        in1=rw.rearrange("p (g o) -> p g o", o=1)
                        .to_broadcast([128, G, E]),
                        op=ALU.mult)

                    # ---- experts ----
                    eo_sb = (None if fast
                             else eopool.tile([128, E, U], F32, name="eo_sb",
                                              tag="eo_sb"))
                    eo_h = (eopool.tile([128, E, U], F16, name="eo_h", tag="eo_h")
                            if fast else None)
                    ct_ps = [ps_c.tile([128, G * 128], F32, name=f"ct{u}",
                                       tag=f"ct{u}")
                             for u in range(2)] if fast else None
                    w_h = None
                    if fast:
                        # fp16 weights for the GPSIMD diagonal builder
                        w_h = gatpool.tile([128, G, E], F16, name="w_h",
                                           tag="w_h", bufs=8)
                        nc.vector.tensor_copy(
                            w_h.rearrange("p g e -> p (g e)"),
                            w_sb.rearrange("p g e -> p (g e)"))
                    xt_dt = F16 if fast else F32
                    W_use = W_r if fast else W_sb
                    xt4_list = []
                    for q in range(E // 4):
                        # 4 transposes into one PSUM bank, one batched copy
                        xt_ps = ps_t.tile([128, 4, 128], F32, name=f"xtp{q}",
                                          tag="xt_ps")
                        for j in range(4):
                            nc.tensor.transpose(
                                xt_ps[:, j, :], x_sb[:, q * 4 + j, :], I_sb[:])
                        xt4 = xtpool.tile([128, 4, 128], xt_dt, name=f"xt4{q}",
                                          tag="xt4")
                        nc.scalar.activation(
                            xt4.rearrange("p j d -> p (j d)"),
                            xt_ps.rearrange("p j d -> p (j d)"), AF.Copy)
                        xt4_list.append(xt4)
                    for q in range(E // 4):
                        for h in range(2):
                            e0 = q * 4 + 2 * h
                            z_ps = ps_e.tile([128, 2, U], F32, name=f"z{e0}",
                                             tag="z_ps")
                            for j in range(2):
                                e = e0 + j
                                nc.tensor.matmul(
                                    z_ps[:, j, :], xt4_list[q][:, 2 * h + j, :],
                                    W_use[:, e, :], start=True, stop=True)
                            if with_expert_bias:
                                nc.vector.tensor_tensor(
                                    out=z_ps.rearrange("p j u -> p (j u)"),
                                    in0=z_ps.rearrange("p j u -> p (j u)"),
                                    in1=ebb[:, e0:e0 + 2, :]
                                    .rearrange("p j u -> p (j u)"),
                                    op=ALU.add)
                            # leaky relu from PSUM, two experts per op;
                            # fast mode writes the fp16 tile directly
                            lk_out = (eo_h if fast else eo_sb)[:, e0:e0 + 2, :]
                            if act_leaky:
                                nc.scalar.activation(
                                    lk_out.rearrange("p j u -> p (j u)"),
                                    z_ps.rearrange("p j u -> p (j u)"),
                                    AF.Prelu, alpha=LEAKY)
                            else:
                                # sim fallback (CoreSim lacks Prelu); two ops
                                tmp = xtpool.tile([128, 2, U], F32,
                                                  tag="lk_tmp")
                                nc.vector.tensor_scalar_mul(
                                    tmp.rearrange("p j u -> p (j u)"),
                                    z_ps.rearrange("p j u -> p (j u)"), LEAKY)
                                nc.vector.tensor_tensor(
                                    out=lk_out.rearrange("p j u -> p (j u)"),
                                    in0=tmp.rearrange("p j u -> p (j u)"),
                                    in1=z_ps.rearrange("p j u -> p (j u)"),
                                    op=ALU.max)
                        nc.sync.dma_start(
                            out=eo_d[row, q * 4:(q + 1) * 4, :],
                            in_=(eo_h if fast else eo_sb)
                            [:, q * 4:(q + 1) * 4, :])
                        if fast:
                            for j in range(4):
                                e = q * 4 + j
                                dw = xtpool.tile([128, G, 128], F16, tag="dw")
                                for g in range(G):
                                    if g < 1 or (g == 1 and e % 2 == 0):
                                        nc.vector.tensor_scalar(
                                            out=dw[:, g, :], in0=I_h[:],
                                            scalar1=w_sb[:, g, e:e + 1],
                                            scalar2=None, op0=ALU.mult)
                                    else:
                                        nc.gpsimd.affine_select(
                                            out=dw[:, g, :],
                                            in_=w_h[:, g, e:e + 1]
                                            .to_broadcast([128, 128]),
                                            compare_op=ALU.is_equal, fill=0.0,
                                            base=0, pattern=[[-1, 128]],
                                            channel_multiplier=1)
                                for uh in range(2):
                                    nc.tensor.matmul(
                                        ct_ps[uh][:],
                                        eo_h[:, e, uh * 128:(uh + 1) * 128],
                                        dw.rearrange("p g b -> p (g b)"),
                                        start=(e == 0), stop=(e == E - 1))

                    if fast:
                        # ct_ps[uh][u, (g, b)] = sum_e w[b,g,e]*eo[b,e,uh*128+u]
                        for uh in range(2):
                            ct_sb = accpool.tile([128, G * 128], F16)
                            nc.scalar.activation(ct_sb[:], ct_ps[uh][:],
                                                 AF.Copy)
                            nc.sync.dma_start(
                                out=co_d[uh, :, :, c * 128:(c + 1) * 128],
                                in_=ct_sb.rearrange("p (g b) -> p g b", g=G))
                    else:
                        # combine on DVE: out[b,g,u] = sum_e w[b,g,e]*eo[b,e,u]
                        acc = accpool.tile([128, G, U], F32)
                        for g in range(G):
                            for e in range(E):
                                w_col = w_sb[:, g, e:e + 1]
                                if e == 0:
                                    nc.vector.tensor_scalar_mul(
                                        acc[:, g, :], eo_sb[:, 0, :], w_col)
                                else:
                                    nc.vector.scalar_tensor_tensor(
                                        out=acc[:, g, :], in0=eo_sb[:, e, :],
                                        scalar=w_col, in1=acc[:, g, :],
                                        op0=ALU.mult, op1=ALU.add)
                        nc.sync.dma_start(
                            out=co_d[row, :],
                            in_=acc.rearrange("p g u -> p (g u)"))
    return nc


_NC_CACHE: dict = {}
FAST = True


def _get_nc(with_eb: bool, with_gb: bool):
    key = (with_eb, with_gb, FAST)
    if key not in _NC_CACHE:
        _NC_CACHE[key] = build_nc(with_eb, with_gb, fast=FAST)
    return _NC_CACHE[key]


def kernel(inputs, feature_input, expert_kernels, expert_biases,
           gating_kernels, gating_biases, global_weights, _trace=False):
    from concourse.bass_utils import run_bass_kernel_spmd

    inputs = np.ascontiguousarray(np.asarray(inputs, dtype=np.float32))
    feature_input = np.ascontiguousarray(np.asarray(feature_input, np.float32))
    ek = np.ascontiguousarray(np.asarray(expert_kernels, np.float32))
    eb = np.asarray(expert_biases, np.float32)
    gk = np.ascontiguousarray(np.asarray(gating_kernels, np.float32))
    gb = np.asarray(gating_biases, np.float32)
    gw = np.ascontiguousarray(np.asarray(global_weights, np.float32))

    with_eb = bool(np.any(eb))
    with_gb = bool(np.any(gb))
    nc = _get_nc(with_eb, with_gb)

    in_maps = []
    for c in range(N_CORES):
        row = slice(c * B_LOC, (c + 1) * B_LOC)
        m = {
            "x": np.ascontiguousarray(inputs[row]),
            "ft": np.ascontiguousarray(feature_input[row]),
            "ek": ek,
            "gk": gk,
            "gw": gw,
        }
        if with_eb:
            m["eb"] = np.ascontiguousarray(eb)
        if with_gb:
            m["gb"] = np.ascontiguousarray(gb)
        in_maps.append(m)

    res = run_bass_kernel_spmd(nc, in_maps, core_ids=list(range(N_CORES)),
                               trace=_trace)
    if FAST:
        # co_t [2, 128, G, B_LOC] -> co [B_LOC, G*256]
        co = np.concatenate(
            [res.results[c]["co"].transpose(3, 2, 0, 1).reshape(B_LOC, G * U)
             for c in range(N_CORES)], axis=0).astype(np.float32)
    else:
        co = np.concatenate([res.results[c]["co"] for c in range(N_CORES)],
                            axis=0)
    eo = np.concatenate([res.results[c]["eo"] for c in range(N_CORES)], axis=0)
    eo = np.ascontiguousarray(eo.transpose(0, 2, 1).astype(np.float32))
    if _trace:
        kernel._last_results = res
    return co, eo
